# revision 1
# baseline (speedup 1.0000x reference)
"""Trainium2 Bass kernel for gated short-time-warp + Conv1d (nn_GW_Conv1D).

Reference computation (per batch element b, C=64 channels, T=32768):
  g = tanh(einsum('ct,c->t', x, est_w)) * 0.5            # velocity, |g| <= 0.5
  d = flow(g)    per 256-window (scaling & squaring, 4 iters), |d| <= 0.5
  xw = interp1d(x, p + d)   per window                    # forward warp
  y = conv1d(xw, conv_w, conv_b, k=3, SAME)               # channel mixing
  d_inv = flow(-g); out = interp1d(y, p + d_inv)          # inverse warp

Because |d| < 1 always (g bounded by tanh*0.5, flow doubles from 1/32 four
times), every linear interpolation touches only nearest neighbours, so the
warps become 3-term elementwise expressions with relu-split coefficients:
  out = x*(1-dn-dp) + x[-1]*dn + x[+1]*dp,  dn=relu(-d), dp=relu(d)
with dn/dp zeroed at window edges (replicating jnp.clip at the borders).

Sharding: pure data parallelism, batch b -> core b (8 cores).

Layouts per core:
  - warp stages: per-channel tiles (128 windows, 256) so the per-(window,w)
    coefficient tiles are shared by all channels with plain tensor_tensor ops.
  - conv: stacked-halves (128 = [ch 0..63 half0; ch 0..63 half1], t) so the
    channel mix is K=128 matmuls with block-diagonal weights.
  - SBUF->SBUF DMAs convert between the two (fp16 staging to save SBUF).
"""
import sys

sys.path.insert(0, "/opt/trn_rl_repo")

import numpy as np
from contextlib import ExitStack

import concourse.bass as bass
import concourse.tile as tile
from concourse import bacc, mybir
from concourse.bass_interp import get_hw_module
from concourse import bass_utils

F32 = mybir.dt.float32
F16 = mybir.dt.float16
AF = mybir.ActivationFunctionType
ALU = mybir.AluOpType

NCORES = 8
C, T, W = 64, 32768, 256
F = T // W            # 128 windows per batch element
H = T // 2            # half length (stacked-halves layout)
FLOW_ITERS = 4
G = 4                 # channels per warp group
NG = C // G


def _flow_iteration(nc, pool, d2, first):
    """One scaling-and-squaring step on d2 (128, 512) = [d_fwd | d_inv].
    d2 <- d2 + interp1d(d2, p + d2), per 256-column window."""
    dn = pool.tile([128, 512], F32, tag="fl_dn")
    dp = pool.tile([128, 512], F32, tag="fl_dp")
    nc.scalar.activation(dn[:], d2[:], AF.Relu, scale=-1.0)
    nc.scalar.activation(dp[:], d2[:], AF.Relu)
    # window-edge masking (jnp.clip at borders)
    nc.gpsimd.memset(dn[:, 0:1], 0.0)
    nc.gpsimd.memset(dn[:, 256:257], 0.0)
    nc.gpsimd.memset(dp[:, 255:256], 0.0)
    nc.gpsimd.memset(dp[:, 511:512], 0.0)
    am = pool.tile([128, 512], F32, tag="fl_am")
    nc.vector.tensor_tensor(am[:], dn[:], dp[:], ALU.add)
    nc.vector.tensor_scalar(am[:], am[:], -1.0, 1.0, ALU.mult, ALU.add)
    itp = pool.tile([128, 512], F32, tag="fl_itp")
    tmp = pool.tile([128, 512], F32, tag="fl_tmp")
    nc.vector.tensor_tensor(itp[:], d2[:], am[:], ALU.mult)
    # left-neighbour term (dn masked at window starts -> cross-window leak is *0)
    nc.vector.tensor_tensor(tmp[:, 1:512], d2[:, 0:511], dn[:, 1:512], ALU.mult)
    nc.vector.tensor_tensor(itp[:, 1:512], itp[:, 1:512], tmp[:, 1:512], ALU.add)
    # right-neighbour term
    nc.vector.tensor_tensor(tmp[:, 0:511], d2[:, 1:512], dp[:, 0:511], ALU.mult)
    nc.vector.tensor_tensor(itp[:, 0:511], itp[:, 0:511], tmp[:, 0:511], ALU.add)
    nc.vector.tensor_tensor(d2[:], d2[:], itp[:], ALU.add)
    return dn, dp


def _build_module():
    nc = bacc.Bacc("TRN2", target_bir_lowering=False, debug=False,
                   enable_asserts=False, num_devices=NCORES)
    x = nc.dram_tensor("x", (C, T), F32, kind="ExternalInput").ap()
    ew = nc.dram_tensor("ew", (128, 2), F32, kind="ExternalInput").ap()
    cw = nc.dram_tensor("cw", (128, 384), F16, kind="ExternalInput").ap()
    cb = nc.dram_tensor("cb", (128, 1), F32, kind="ExternalInput").ap()
    y = nc.dram_tensor("y", (C, T), F32, kind="ExternalOutput").ap()

    x_hc = x.rearrange("c (h t) -> h c t", h=2)          # (2, 64, H)
    x_fc = x.rearrange("c (f w) -> f c w", w=W)          # (128, 64, 256)
    y_cf = y.rearrange("c (f w) -> c f w", w=W)          # (64, 128, 256)

    with tile.TileContext(nc) as tc, ExitStack() as ctx:
        big = ctx.enter_context(tc.tile_pool(name="big", bufs=1))
        coef = ctx.enter_context(tc.tile_pool(name="coef", bufs=1))
        sm = ctx.enter_context(tc.tile_pool(name="sm", bufs=1))

        # persistent big buffers (fp16 staging for the conv layout)
        xw_st = big.tile([128, H + 2], F16)   # col 0 / col H+1 = conv halo
        yc_st = big.tile([128, H], F16)

        # ---------------- Stage A: g = einsum(x, est_w) --------------------
        ew_sb = sm.tile([128, 2], F32, tag="ew")
        nc.sync.dma_start(ew_sb[:], ew)
        g_cmp = coef.tile([128, W], F32)      # g, windows on partitions
        with tc.tile_pool(name="eins", bufs=2) as eins, \
             tc.tile_pool(name="psA", bufs=2, space="PSUM") as psA:
            for i in range(0, H, 2048):
                xe = eins.tile([128, 2048], F32, tag="xe")
                nc.sync.dma_start(xe[:], x_hc[:, :, i:i + 2048])
                pg = psA.tile([2, 2048], F32, tag="pg")
                for j in range(4):
                    nc.tensor.matmul(pg[:, j * 512:(j + 1) * 512], ew_sb[:],
                                     xe[:, j * 512:(j + 1) * 512],
                                     start=True, stop=True)
                gch = eins.tile([2, 2048], F32, tag="gch")
                nc.scalar.copy(gch[:], pg[:])
                r = i // W
                nc.sync.dma_start(g_cmp[r:r + 8, :], gch[0:1, :])
                nc.sync.dma_start(g_cmp[64 + r:64 + r + 8, :], gch[1:2, :])

        psB = ctx.enter_context(tc.tile_pool(name="psB", bufs=4, space="PSUM"))

        # ---------------- Stage B: flow + warp coefficients ----------------
        g_th = sm.tile([128, W], F32, tag="gth")
        nc.scalar.activation(g_th[:], g_cmp[:], AF.Tanh)
        d2 = sm.tile([128, 512], F32, tag="d2")           # [d_fwd | d_inv]
        nc.vector.tensor_scalar_mul(d2[:, 0:256], g_th[:], 0.5 / 16.0)
        nc.vector.tensor_scalar_mul(d2[:, 256:512], g_th[:], -0.5 / 16.0)
        for it in range(FLOW_ITERS):
            dn, dp = _flow_iteration(nc, sm, d2, it == 0)
        # final coefficients from the integrated displacement
        dn = sm.tile([128, 512], F32, tag="cf_dn")
        dp = sm.tile([128, 512], F32, tag="cf_dp")
        nc.scalar.activation(dn[:], d2[:], AF.Relu, scale=-1.0)
        nc.scalar.activation(dp[:], d2[:], AF.Relu)
        nc.gpsimd.memset(dn[:, 0:1], 0.0)
        nc.gpsimd.memset(dn[:, 256:257], 0.0)
        nc.gpsimd.memset(dp[:, 255:256], 0.0)
        nc.gpsimd.memset(dp[:, 511:512], 0.0)
        am = sm.tile([128, 512], F32, tag="cf_am")
        nc.vector.tensor_tensor(am[:], dn[:], dp[:], ALU.add)
        nc.vector.tensor_scalar(am[:], am[:], -1.0, 1.0, ALU.mult, ALU.add)

        # replicate each coefficient G times along free dim (group tiles)
        GW = G * W
        big_coefs = {}
        for name, src, off in (("af", am, 0), ("dnf", dn, 0), ("dpf", dp, 0),
                               ("ai", am, 256), ("dni", dn, 256), ("dpi", dp, 256)):
            t = coef.tile([128, GW], F32, tag=name)
            nc.scalar.copy(t[:, 0:W], src[:, off:off + W])
            rep = W
            while rep < GW:
                nc.scalar.copy(t[:, rep:2 * rep], t[:, 0:rep])
                rep *= 2
            big_coefs[name] = t
        af, dnf, dpf = big_coefs["af"], big_coefs["dnf"], big_coefs["dpf"]
        ai, dni, dpi = big_coefs["ai"], big_coefs["dni"], big_coefs["dpi"]

        # ---------------- Stage C: forward warp ----------------------------
        wrk = ctx.enter_context(tc.tile_pool(name="wrk", bufs=3))
        for g in range(NG):
            c0 = g * G
            xg = wrk.tile([128, GW], F32, tag="win")
            nc.sync.dma_start(xg[:], x_fc[:, c0:c0 + G, :])
            xw = wrk.tile([128, GW], F32, tag="wout")
            tmp = wrk.tile([128, GW], F32, tag="wtmp")
            nc.vector.tensor_tensor(xw[:], xg[:], af[:], ALU.mult)
            nc.vector.tensor_tensor(tmp[:, 1:GW], xg[:, 0:GW - 1],
                                    dnf[:, 1:GW], ALU.mult)
            nc.vector.tensor_tensor(xw[:, 1:GW], xw[:, 1:GW],
                                    tmp[:, 1:GW], ALU.add)
            nc.vector.tensor_tensor(tmp[:, 0:GW - 1], xg[:, 1:GW],
                                    dpf[:, 0:GW - 1], ALU.mult)
            nc.vector.tensor_tensor(xw[:, 0:GW - 1], xw[:, 0:GW - 1],
                                    tmp[:, 0:GW - 1], ALU.add)
            for cl in range(G):
                c = c0 + cl
                nc.gpsimd.dma_start(xw_st[c:c + 1, 1:H + 1],
                                    xw[0:64, cl * W:(cl + 1) * W])
                nc.gpsimd.dma_start(xw_st[64 + c:65 + c, 1:H + 1],
                                    xw[64:128, cl * W:(cl + 1) * W])

        # conv halo columns: half0 left pad = 0, half1 right pad = 0,
        # cross-half continuity for the interior boundary
        nc.gpsimd.memset(xw_st[0:64, 0:1], 0.0)
        nc.gpsimd.memset(xw_st[64:128, H + 1:H + 2], 0.0)
        nc.sync.dma_start(xw_st[64:128, 0:1], xw_st[0:64, H:H + 1])
        nc.sync.dma_start(xw_st[0:64, H + 1:H + 2], xw_st[64:128, 1:2])

        # ---------------- Stage D: Conv1d(C,C,3,SAME) -----------------------
        cw_sb = sm.tile([128, 384], F16, tag="cw")
        nc.sync.dma_start(cw_sb[:], cw)
        cb_sb = sm.tile([128, 1], F32, tag="cb")
        nc.sync.dma_start(cb_sb[:], cb)
        for k in range(H // 512):
            pc = psB.tile([128, 512], F32, tag="pc")
            for j in range(3):
                nc.tensor.matmul(pc[:], cw_sb[:, j * 128:(j + 1) * 128],
                                 xw_st[:, k * 512 + j:k * 512 + j + 512],
                                 start=(j == 0), stop=(j == 2))
            nc.scalar.activation(yc_st[:, k * 512:(k + 1) * 512], pc[:],
                                 AF.Identity, bias=cb_sb[:])

        # ---------------- Stage E: inverse warp + store ---------------------
        for g in range(NG):
            c0 = g * G
            yg = wrk.tile([128, GW], F32, tag="win")
            for cl in range(G):
                c = c0 + cl
                nc.gpsimd.dma_start(yg[0:64, cl * W:(cl + 1) * W],
                                    yc_st[c:c + 1, :])
                nc.gpsimd.dma_start(yg[64:128, cl * W:(cl + 1) * W],
                                    yc_st[64 + c:65 + c, :])
            yo = wrk.tile([128, GW], F32, tag="wout")
            tmp = wrk.tile([128, GW], F32, tag="wtmp")
            nc.vector.tensor_tensor(yo[:], yg[:], ai[:], ALU.mult)
            nc.vector.tensor_tensor(tmp[:, 1:GW], yg[:, 0:GW - 1],
                                    dni[:, 1:GW], ALU.mult)
            nc.vector.tensor_tensor(yo[:, 1:GW], yo[:, 1:GW],
                                    tmp[:, 1:GW], ALU.add)
            nc.vector.tensor_tensor(tmp[:, 0:GW - 1], yg[:, 1:GW],
                                    dpi[:, 0:GW - 1], ALU.mult)
            nc.vector.tensor_tensor(yo[:, 0:GW - 1], yo[:, 0:GW - 1],
                                    tmp[:, 0:GW - 1], ALU.add)
            for cl in range(G):
                nc.sync.dma_start(y_cf[c0 + cl], yo[:, cl * W:(cl + 1) * W])

    nc.compile()
    return nc


def _host_params(est_w, conv_w, conv_b):
    ew = np.zeros((128, 2), np.float32)
    ew[:64, 0] = est_w
    ew[64:, 1] = est_w
    cw = np.zeros((128, 384), np.float16)
    for j in range(3):
        blk = conv_w[:, :, j].T.astype(np.float16)   # (in, out)
        cw[0:64, j * 128:j * 128 + 64] = blk
        cw[64:128, j * 128 + 64:j * 128 + 128] = blk
    cb = np.concatenate([conv_b, conv_b]).astype(np.float32)[:, None]
    return ew, cw, cb


_COMPILED = None


def _get_compiled():
    global _COMPILED
    if _COMPILED is None:
        nc = _build_module()
        nc.m = get_hw_module(nc.m)
        _COMPILED = nc
    return _COMPILED


def kernel(signal, est_w, conv_w, conv_b, _trace=False, _trace_kwargs=None):
    nc = _get_compiled()
    ew, cw, cb = _host_params(np.asarray(est_w, np.float32),
                              np.asarray(conv_w, np.float32),
                              np.asarray(conv_b, np.float32))
    signal = np.ascontiguousarray(np.asarray(signal, np.float32))
    in_maps = [{"x": signal[b], "ew": ew, "cw": cw, "cb": cb}
               for b in range(NCORES)]
    res = bass_utils.run_bass_kernel_spmd(
        nc, in_maps, core_ids=list(range(NCORES)), trace=_trace,
        **(_trace_kwargs or {}))
    out = np.stack([r["y"] for r in res.results], axis=0)
    if _trace:
        return out, res
    return out

